# revision 6
# baseline (speedup 1.0000x reference)
"""Trainium2 Bass kernel for packed-sequence affine + ragged mean-pool.

reference semantics:
    y = alpha * x + bias                      # [T, D] elementwise over D
    sums = segment_sum(y, seq_ids, B)         # [B, D]
    counts = segment_sum(ones, seq_ids, B)    # [B]
    pooled = sums / counts[:, None]           # [B, D]
    return (pooled, None)

Strategy (8 NeuronCores, data-parallel over tokens):
  - Shard tokens contiguously across cores (padded so every core gets the
    same shape; pad tokens carry seq_id = B which one-hot-misses everything).
  - x is split on the host into bf16 hi/lo planes (x == hi + lo to ~2^-16
    relative): TensorE runs bf16 matmuls at full rate (fp32 matmuls cost 4x),
    and since bf16*bf16 products are exact in fp32 PSUM the only error is the
    hi+lo residual.
  - Per core, per 256-token tile: DMA [128p, 2u, 2c, D] (1 MiB), build a
    one-hot [128, u, B] from seq_ids via iota + is_equal, and accumulate
    sums[B, D] in PSUM with TensorE matmuls (segment_sum == onehot.T @ x).
  - alpha/bias/counts fold into a host-prepared per-core postprocess:
        out_core = psum * (alpha/counts) + (bias/8)
    so summing over cores on the host gives
        pooled = alpha * segsum(x)/counts + bias = segsum(alpha*x+bias)/counts.
  - Host gathers: pooled = sum over cores of out_core.
"""

import math
import os

import numpy as np

P = 128          # SBUF partitions
U = 2            # token groups per partition per tile (256-token, 1 MiB tiles)
N_CORES = 8
MM_N = 512       # PSUM-bank limit for f32 accumulation

_cache = {}

# test.py support: when BASS_KERNEL_TRACE=1, run with NTFF profiling and stash
# the BassKernelResults here.
last_results = None


def _build(tok_pad, B, D):
    import concourse.mybir as mybir
    import concourse.tile as tile
    from concourse import bacc

    f32 = mybir.dt.float32
    bf16 = mybir.dt.bfloat16

    n_tiles = tok_pad // (P * U)
    assert tok_pad == n_tiles * P * U
    assert D % MM_N == 0
    n_halves = D // MM_N

    nc = bacc.Bacc()
    # hi/lo planes interleaved per token: [tok, c, D] with c in {hi, lo}
    x_in = nc.dram_tensor("xhl", [tok_pad, 2, D], bf16, kind="ExternalInput")
    seq_in = nc.dram_tensor("seq", [tok_pad], bf16, kind="ExternalInput")
    scale_in = nc.dram_tensor("scale", [B, D], f32, kind="ExternalInput")
    biasr_in = nc.dram_tensor("biasr", [B, D], f32, kind="ExternalInput")
    out = nc.dram_tensor("out", [B, D], f32, kind="ExternalOutput")

    xv = x_in.rearrange("(t p u) c d -> t p u c d", p=P, u=U)
    seqv = seq_in.rearrange("(t p u) -> t p u", p=P, u=U)

    with tile.TileContext(nc) as tc:
        with (
            tc.tile_pool(name="xp", bufs=8) as xp,
            tc.tile_pool(name="sp", bufs=8) as sp,
            tc.tile_pool(name="cp", bufs=1) as cp,
            tc.tile_pool(name="pp", bufs=1, space="PSUM") as pp,
        ):
            iota_t = cp.tile([P, B], bf16)
            nc.gpsimd.iota(
                iota_t[:],
                pattern=[[1, B]],
                base=0,
                channel_multiplier=0,
                allow_small_or_imprecise_dtypes=True,
            )

            psum_t = pp.tile([B, D], f32)
            scale_t = cp.tile([B, D], f32)
            biasr_t = cp.tile([B, D], f32)

            for t in range(n_tiles):
                st = sp.tile([P, U], bf16)
                nc.sync.dma_start(st[:], seqv[t])
                oh = sp.tile([P, U, B], bf16)
                nc.vector.tensor_tensor(
                    oh[:],
                    st[:, :, None].to_broadcast([P, U, B]),
                    iota_t[:, None, :].to_broadcast([P, U, B]),
                    mybir.AluOpType.is_equal,
                )
                xt = xp.tile([P, U, 2, D], bf16)
                # alternate between the two HWDGE rings (SP / ACT)
                dma_eng = nc.sync if t % 2 == 0 else nc.scalar
                dma_eng.dma_start(xt[:], xv[t])
                if t == 1:
                    # postprocess constants: load early enough to overlap, but
                    # after the first x tile so they don't delay the pipeline
                    nc.scalar.dma_start(scale_t[:], scale_in[:])
                    nc.scalar.dma_start(biasr_t[:], biasr_in[:])
                for u in range(U):
                    for c in range(2):
                        for h in range(n_halves):
                            nc.tensor.matmul(
                                psum_t[:, h * MM_N:(h + 1) * MM_N],
                                lhsT=oh[:, u, :],
                                rhs=xt[:, u, c, h * MM_N:(h + 1) * MM_N],
                                start=(t == 0 and u == 0 and c == 0),
                                stop=(t == n_tiles - 1 and u == U - 1 and c == 1),
                            )

            out_t = cp.tile([B, D], f32)
            nc.vector.tensor_mul(out_t[:], psum_t[:], scale_t[:])
            nc.vector.tensor_add(out_t[:], out_t[:], biasr_t[:])
            nc.sync.dma_start(out[:], out_t[:])

    nc.compile()
    return nc


def kernel(x_data, alpha, bias, seq_ids, batch_size):
    global last_results
    import ml_dtypes

    bf16 = ml_dtypes.bfloat16

    x = np.asarray(x_data, dtype=np.float32)
    alpha = np.asarray(alpha, dtype=np.float32)
    bias = np.asarray(bias, dtype=np.float32)
    seq = np.asarray(seq_ids)
    B = int(batch_size)
    T, D = x.shape

    tile_tok = P * U
    tok_pad = math.ceil(T / (N_CORES * tile_tok)) * tile_tok
    total_pad = tok_pad * N_CORES

    # bf16 hi/lo split: x == hi + lo up to ~2^-16 relative
    xhl = np.zeros((N_CORES * tok_pad, 2, D), dtype=bf16)
    hi = x.astype(bf16)
    xhl[:T, 0, :] = hi
    xhl[:T, 1, :] = (x - hi.astype(np.float32)).astype(bf16)
    xhl = xhl.reshape(N_CORES, tok_pad, 2, D)

    # pad tokens get seq id B: the one-hot row is all-zero so they add nothing
    seq_pad = np.full((N_CORES * tok_pad), float(B), dtype=bf16)
    seq_pad[:T] = seq.astype(np.int32).astype(bf16)  # ids < 256 are exact in bf16
    seq_pad = seq_pad.reshape(N_CORES, tok_pad)

    counts = np.bincount(np.asarray(seq, dtype=np.int64), minlength=B).astype(np.float64)
    scale = (alpha.astype(np.float64)[None, :] / counts[:, None]).astype(np.float32)
    biasr = np.broadcast_to((bias.astype(np.float64) / N_CORES).astype(np.float32),
                            (B, D)).copy()

    key = (tok_pad, B, D)
    if key not in _cache:
        _cache[key] = _build(tok_pad, B, D)
    nc = _cache[key]

    in_maps = [
        {"xhl": xhl[c], "seq": seq_pad[c], "scale": scale, "biasr": biasr}
        for c in range(N_CORES)
    ]

    trace = os.environ.get("BASS_KERNEL_TRACE", "") == "1"
    run_kwargs = {}
    if trace:
        import sys
        import types

        import antenv
        from trn_agent_boot.trn_boot import _ntff_profile_via_ctypes

        if "antenv.axon_hooks" not in sys.modules:
            mod = types.ModuleType("antenv.axon_hooks")
            hook = _ntff_profile_via_ctypes("/opt/axon/libaxon_pjrt.so")
            mod.get_axon_ntff_profile_hook = lambda: hook
            sys.modules["antenv.axon_hooks"] = mod
            antenv.axon_hooks = mod
        from concourse import bass_utils as _bu

        _bu.upload_artifacts = lambda tmpdir: "local://skipped"
        run_kwargs = {"trace": True, "trace_cores": [0]}

    from concourse.bass_utils import run_bass_kernel_spmd

    res = run_bass_kernel_spmd(nc, in_maps, core_ids=list(range(N_CORES)),
                               **run_kwargs)
    last_results = res

    pooled = np.zeros((B, D), dtype=np.float32)
    for r in res.results:
        pooled += r["out"]
    return (pooled, None)


# revision 14
# speedup vs baseline: 1.0037x; 1.0037x over previous
"""Trainium2 Bass kernel for packed-sequence affine + ragged mean-pool.

reference semantics:
    y = alpha * x + bias                      # [T, D] elementwise over D
    sums = segment_sum(y, seq_ids, B)         # [B, D]
    counts = segment_sum(ones, seq_ids, B)    # [B]
    pooled = sums / counts[:, None]           # [B, D]
    return (pooled, None)

Strategy (8 NeuronCores, data-parallel over tokens):
  - Shard tokens contiguously across cores (padded so every core gets the
    same shape; pad tokens carry seq_id = B which one-hot-misses everything).
  - x is split on the host into bf16 hi/lo planes (x == hi + lo to ~2^-16
    relative): TensorE runs bf16 matmuls at full rate (fp32 matmuls cost 4x),
    and since bf16*bf16 products are exact in fp32 PSUM the only error is the
    hi+lo residual.
  - Per core, per 256-token tile: DMA [128p, 2u, 2c, D] (1 MiB), build a
    one-hot [128, u, B] from seq_ids via iota + is_equal, and accumulate
    sums[B, D] in PSUM with TensorE matmuls (segment_sum == onehot.T @ x).
  - alpha/bias/counts fold into a host-prepared per-core postprocess:
        out_core = psum * (alpha/counts) + (bias/8)
    so summing over cores on the host gives
        pooled = alpha * segsum(x)/counts + bias = segsum(alpha*x+bias)/counts.
  - Host gathers: pooled = sum over cores of out_core.
"""

import math
import os

import numpy as np

P = 128          # SBUF partitions
U = 2            # token groups per partition per tile (256-token, 1 MiB tiles)
N_CORES = 8
MM_N = 512       # PSUM-bank limit for f32 accumulation

_cache = {}

# test.py support: when BASS_KERNEL_TRACE=1, run with NTFF profiling and stash
# the BassKernelResults here.
last_results = None


def _build(tok_pad, B, D, has_bias):
    import concourse.mybir as mybir
    import concourse.tile as tile
    from concourse import bacc

    f32 = mybir.dt.float32
    bf16 = mybir.dt.bfloat16

    n_full = tok_pad // (P * U)
    rem = tok_pad - n_full * P * U       # leftover tokens, multiple of U
    p_rem = rem // U
    assert rem == p_rem * U
    assert D % MM_N == 0
    n_halves = D // MM_N

    nc = bacc.Bacc()
    # hi/lo planes interleaved per token: [tok, c, D] with c in {hi, lo}
    x_in = nc.dram_tensor("xhl", [tok_pad, 2, D], bf16, kind="ExternalInput")
    seq_in = nc.dram_tensor("seq", [tok_pad], bf16, kind="ExternalInput")
    scale_in = nc.dram_tensor("scale", [B, D], f32, kind="ExternalInput")
    biasr_in = (nc.dram_tensor("biasr", [B, D], f32, kind="ExternalInput")
                if has_bias else None)
    out = nc.dram_tensor("out", [B, D], f32, kind="ExternalOutput")

    full_tok = n_full * P * U
    xv = x_in[:full_tok].rearrange("(t p u) c d -> t p u c d", p=P, u=U)
    seqv = seq_in[:full_tok].rearrange("(t p u) -> t p u", p=P, u=U)
    if rem:
        xr = x_in[full_tok:].rearrange("(p u) c d -> p u c d", u=U)
        seqr = seq_in[full_tok:].rearrange("(p u) -> p u", u=U)

    with tile.TileContext(nc) as tc:
        with (
            tc.tile_pool(name="xp", bufs=8) as xp,
            tc.tile_pool(name="sp", bufs=8) as sp,
            tc.tile_pool(name="cp", bufs=1) as cp,
            tc.tile_pool(name="pp", bufs=1, space="PSUM") as pp,
        ):
            iota_t = cp.tile([P, B], bf16)
            nc.gpsimd.iota(
                iota_t[:],
                pattern=[[1, B]],
                base=0,
                channel_multiplier=0,
                allow_small_or_imprecise_dtypes=True,
            )

            psum_t = pp.tile([B, D], f32)
            scale_t = cp.tile([B, D], f32)
            biasr_t = cp.tile([B, D], f32, name="biasr_t") if has_bias else None

            for t in range(n_full):
                st = sp.tile([P, U], bf16)
                nc.sync.dma_start(st[:], seqv[t])
                oh = sp.tile([P, U, B], bf16)
                nc.vector.tensor_tensor(
                    oh[:],
                    st[:, :, None].to_broadcast([P, U, B]),
                    iota_t[:, None, :].to_broadcast([P, U, B]),
                    mybir.AluOpType.is_equal,
                )
                xt = xp.tile([P, U, 2, D], bf16)
                # alternate between the two HWDGE rings (SP / ACT)
                dma_eng = nc.sync if t % 2 == 0 else nc.scalar
                dma_eng.dma_start(xt[:], xv[t])
                if t == 1:
                    # postprocess constants: load early enough to overlap, but
                    # after the first x tile so they don't delay the pipeline
                    nc.scalar.dma_start(scale_t[:], scale_in[:])
                    if has_bias:
                        nc.scalar.dma_start(biasr_t[:], biasr_in[:])
                last = rem == 0 and t == n_full - 1
                for u in range(U):
                    for c in range(2):
                        for h in range(n_halves):
                            nc.tensor.matmul(
                                psum_t[:, h * MM_N:(h + 1) * MM_N],
                                lhsT=oh[:, u, :],
                                rhs=xt[:, u, c, h * MM_N:(h + 1) * MM_N],
                                start=(t == 0 and u == 0 and c == 0),
                                stop=(last and u == U - 1 and c == 1),
                            )

            if rem:
                st = sp.tile([p_rem, U], bf16)
                nc.sync.dma_start(st[:], seqr[:])
                oh = sp.tile([p_rem, U, B], bf16)
                nc.vector.tensor_tensor(
                    oh[:],
                    st[:, :, None].to_broadcast([p_rem, U, B]),
                    iota_t[:p_rem, None, :].to_broadcast([p_rem, U, B]),
                    mybir.AluOpType.is_equal,
                )
                xt = xp.tile([p_rem, U, 2, D], bf16, tag="xrem")
                nc.sync.dma_start(xt[:], xr[:])
                for u in range(U):
                    for c in range(2):
                        for h in range(n_halves):
                            nc.tensor.matmul(
                                psum_t[:, h * MM_N:(h + 1) * MM_N],
                                lhsT=oh[:, u, :],
                                rhs=xt[:, u, c, h * MM_N:(h + 1) * MM_N],
                                start=(n_full == 0 and u == 0 and c == 0),
                                stop=(u == U - 1 and c == 1),
                            )

            out_t = cp.tile([B, D], f32)
            nc.vector.tensor_mul(out_t[:], psum_t[:], scale_t[:])
            if has_bias:
                nc.vector.tensor_add(out_t[:], out_t[:], biasr_t[:])
            nc.sync.dma_start(out[:], out_t[:])

    nc.compile()
    return nc


def kernel(x_data, alpha, bias, seq_ids, batch_size):
    global last_results
    import ml_dtypes

    bf16 = ml_dtypes.bfloat16

    x = np.asarray(x_data, dtype=np.float32)
    alpha = np.asarray(alpha, dtype=np.float32)
    bias = np.asarray(bias, dtype=np.float32)
    seq = np.asarray(seq_ids)
    B = int(batch_size)
    T, D = x.shape

    assert B <= P, f"segment count {B} must fit in {P} PSUM partitions"
    # pad only to a multiple of U tokens per core; a sub-128-partition
    # remainder tile handles the last few tokens
    tok_pad = math.ceil(T / (N_CORES * U)) * U

    # bf16 hi/lo split: x == hi + lo up to ~2^-16 relative
    xhl = np.zeros((N_CORES * tok_pad, 2, D), dtype=bf16)
    hi = x.astype(bf16)
    xhl[:T, 0, :] = hi
    xhl[:T, 1, :] = (x - hi.astype(np.float32)).astype(bf16)
    xhl = xhl.reshape(N_CORES, tok_pad, 2, D)

    # pad tokens get seq id B: the one-hot row is all-zero so they add nothing
    seq_pad = np.full((N_CORES * tok_pad), float(B), dtype=bf16)
    seq_pad[:T] = seq.astype(np.int32).astype(bf16)  # ids < 256 are exact in bf16
    seq_pad = seq_pad.reshape(N_CORES, tok_pad)

    seq_i = np.asarray(seq, dtype=np.int64)
    counts = np.bincount(seq_i[(seq_i >= 0) & (seq_i < B)],
                         minlength=B)[:B].astype(np.float64)
    scale = (alpha.astype(np.float64)[None, :] / counts[:, None]).astype(np.float32)
    biasr = np.broadcast_to((bias.astype(np.float64) / N_CORES).astype(np.float32),
                            (B, D)).copy()

    has_bias = bool(np.any(biasr))
    key = (tok_pad, B, D, has_bias)
    if key not in _cache:
        _cache[key] = _build(tok_pad, B, D, has_bias)
    nc = _cache[key]

    in_maps = []
    for c in range(N_CORES):
        m = {"xhl": xhl[c], "seq": seq_pad[c], "scale": scale}
        if has_bias:
            m["biasr"] = biasr
        in_maps.append(m)

    trace = os.environ.get("BASS_KERNEL_TRACE", "") == "1"
    run_kwargs = {}
    if trace:
        import sys
        import types

        import antenv
        from trn_agent_boot.trn_boot import _ntff_profile_via_ctypes

        if "antenv.axon_hooks" not in sys.modules:
            mod = types.ModuleType("antenv.axon_hooks")
            hook = _ntff_profile_via_ctypes("/opt/axon/libaxon_pjrt.so")
            mod.get_axon_ntff_profile_hook = lambda: hook
            sys.modules["antenv.axon_hooks"] = mod
            antenv.axon_hooks = mod
        from concourse import bass_utils as _bu

        _bu.upload_artifacts = lambda tmpdir: "local://skipped"
        run_kwargs = {"trace": True, "trace_cores": [0]}

    from concourse.bass_utils import run_bass_kernel_spmd

    res = run_bass_kernel_spmd(nc, in_maps, core_ids=list(range(N_CORES)),
                               **run_kwargs)
    last_results = res

    pooled = np.zeros((B, D), dtype=np.float32)
    for r in res.results:
        pooled += r["out"]
    return (pooled, None)


# revision 16
# speedup vs baseline: 1.0789x; 1.0749x over previous
"""Trainium2 Bass kernel for packed-sequence affine + ragged mean-pool.

reference semantics:
    y = alpha * x + bias                      # [T, D] elementwise over D
    sums = segment_sum(y, seq_ids, B)         # [B, D]
    counts = segment_sum(ones, seq_ids, B)    # [B]
    pooled = sums / counts[:, None]           # [B, D]
    return (pooled, None)

Strategy (8 NeuronCores, data-parallel over tokens):
  - Shard tokens contiguously across cores (padded so every core gets the
    same shape; pad tokens carry seq_id = B which one-hot-misses everything).
  - x is split on the host into bf16 hi/lo planes (x == hi + lo to ~2^-16
    relative): TensorE runs bf16 matmuls at full rate (fp32 matmuls cost 4x),
    and since bf16*bf16 products are exact in fp32 PSUM the only error is the
    hi+lo residual.
  - Per core, per 256-token tile: DMA [128p, 2u, 2c, D] (1 MiB), build a
    one-hot [128, u, B] from seq_ids via iota + is_equal, and accumulate
    sums[B, D] in PSUM with TensorE matmuls (segment_sum == onehot.T @ x).
  - alpha/bias/counts fold into a host-prepared per-core postprocess:
        out_core = psum * (alpha/counts) + (bias/8)
    so summing over cores on the host gives
        pooled = alpha * segsum(x)/counts + bias = segsum(alpha*x+bias)/counts.
  - Host gathers: pooled = sum over cores of out_core.
"""

import math
import os

import numpy as np

P = 128          # SBUF partitions
U = 2            # token groups per partition per tile (256-token, 1 MiB tiles)
N_CORES = 8
MM_N = 512       # PSUM-bank limit for f32 accumulation

_cache = {}

# test.py support: when BASS_KERNEL_TRACE=1, run with NTFF profiling and stash
# the BassKernelResults here.
last_results = None


def _build(tok_pad, B, D, has_bias):
    import concourse.mybir as mybir
    import concourse.tile as tile
    from concourse import bacc

    f32 = mybir.dt.float32
    bf16 = mybir.dt.bfloat16

    n_full = tok_pad // (P * U)
    rem = tok_pad - n_full * P * U       # leftover tokens, multiple of U
    p_rem = rem // U
    assert rem == p_rem * U
    assert D % MM_N == 0
    n_halves = D // MM_N

    nc = bacc.Bacc()
    # hi/lo planes interleaved per token: [tok, c, D] with c in {hi, lo}
    x_in = nc.dram_tensor("xhl", [tok_pad, 2, D], bf16, kind="ExternalInput")
    seq_in = nc.dram_tensor("seq", [tok_pad], bf16, kind="ExternalInput")
    scale_in = nc.dram_tensor("scale", [B, D], f32, kind="ExternalInput")
    biasr_in = (nc.dram_tensor("biasr", [B, D], f32, kind="ExternalInput")
                if has_bias else None)
    out = nc.dram_tensor("out", [B, D], f32, kind="ExternalOutput")

    full_tok = n_full * P * U
    xv = x_in[:full_tok].rearrange("(t p u) c d -> t p u c d", p=P, u=U)
    seqv = seq_in[:full_tok].rearrange("(t p u) -> t p u", p=P, u=U)
    if rem:
        xr = x_in[full_tok:].rearrange("(p u) c d -> p u c d", u=U)
        seqr = seq_in[full_tok:].rearrange("(p u) -> p u", u=U)

    with tile.TileContext(nc) as tc:
        with (
            tc.tile_pool(name="xp", bufs=8) as xp,
            tc.tile_pool(name="sp", bufs=8) as sp,
            tc.tile_pool(name="cp", bufs=1) as cp,
            tc.tile_pool(name="pp", bufs=1, space="PSUM") as pp,
        ):
            iota_t = cp.tile([P, B], bf16)
            nc.gpsimd.iota(
                iota_t[:],
                pattern=[[1, B]],
                base=0,
                channel_multiplier=0,
                allow_small_or_imprecise_dtypes=True,
            )

            psum_t = pp.tile([B, D], f32)
            scale_t = cp.tile([B, D], f32)
            biasr_t = cp.tile([B, D], f32, name="biasr_t") if has_bias else None

            for t in range(n_full):
                st = sp.tile([P, U], bf16)
                nc.sync.dma_start(st[:], seqv[t])
                oh = sp.tile([P, U, B], bf16)
                nc.vector.tensor_tensor(
                    oh[:],
                    st[:, :, None].to_broadcast([P, U, B]),
                    iota_t[:, None, :].to_broadcast([P, U, B]),
                    mybir.AluOpType.is_equal,
                )
                xt = xp.tile([P, U, 2, D], bf16)
                # alternate between the two HWDGE rings (SP / ACT)
                dma_eng = nc.sync if t % 2 == 0 else nc.scalar
                dma_eng.dma_start(xt[:], xv[t])
                if t == 1:
                    # postprocess constants: load early enough to overlap, but
                    # after the first x tile so they don't delay the pipeline
                    nc.scalar.dma_start(scale_t[:], scale_in[:])
                    if has_bias:
                        nc.scalar.dma_start(biasr_t[:], biasr_in[:])
                last = rem == 0 and t == n_full - 1
                for u in range(U):
                    for c in range(2):
                        for h in range(n_halves):
                            nc.tensor.matmul(
                                psum_t[:, h * MM_N:(h + 1) * MM_N],
                                lhsT=oh[:, u, :],
                                rhs=xt[:, u, c, h * MM_N:(h + 1) * MM_N],
                                start=(t == 0 and u == 0 and c == 0),
                                stop=(last and u == U - 1 and c == 1),
                            )

            if rem:
                st = cp.tile([p_rem, U], bf16, name="st_rem")
                nc.sync.dma_start(st[:], seqr[:])
                oh = cp.tile([p_rem, U, B], bf16, name="oh_rem")
                nc.vector.tensor_tensor(
                    oh[:],
                    st[:, :, None].to_broadcast([p_rem, U, B]),
                    iota_t[:p_rem, None, :].to_broadcast([p_rem, U, B]),
                    mybir.AluOpType.is_equal,
                )
                xt = cp.tile([p_rem, U, 2, D], bf16, name="xrem")
                nc.sync.dma_start(xt[:], xr[:])
                for u in range(U):
                    for c in range(2):
                        for h in range(n_halves):
                            nc.tensor.matmul(
                                psum_t[:, h * MM_N:(h + 1) * MM_N],
                                lhsT=oh[:, u, :],
                                rhs=xt[:, u, c, h * MM_N:(h + 1) * MM_N],
                                start=(n_full == 0 and u == 0 and c == 0),
                                stop=(u == U - 1 and c == 1),
                            )

            out_t = cp.tile([B, D], f32)
            nc.vector.tensor_mul(out_t[:], psum_t[:], scale_t[:])
            if has_bias:
                nc.vector.tensor_add(out_t[:], out_t[:], biasr_t[:])
            nc.sync.dma_start(out[:], out_t[:])

    nc.compile()
    return nc


def kernel(x_data, alpha, bias, seq_ids, batch_size):
    global last_results
    import ml_dtypes

    bf16 = ml_dtypes.bfloat16

    x = np.asarray(x_data, dtype=np.float32)
    alpha = np.asarray(alpha, dtype=np.float32)
    bias = np.asarray(bias, dtype=np.float32)
    seq = np.asarray(seq_ids)
    B = int(batch_size)
    T, D = x.shape

    assert B <= P, f"segment count {B} must fit in {P} PSUM partitions"
    # pad only to a multiple of U tokens per core; a sub-128-partition
    # remainder tile handles the last few tokens
    tok_pad = math.ceil(T / (N_CORES * U)) * U

    # bf16 hi/lo split: x == hi + lo up to ~2^-16 relative
    xhl = np.zeros((N_CORES * tok_pad, 2, D), dtype=bf16)
    hi = x.astype(bf16)
    xhl[:T, 0, :] = hi
    xhl[:T, 1, :] = (x - hi.astype(np.float32)).astype(bf16)
    xhl = xhl.reshape(N_CORES, tok_pad, 2, D)

    # pad tokens get seq id B: the one-hot row is all-zero so they add nothing
    seq_pad = np.full((N_CORES * tok_pad), float(B), dtype=bf16)
    seq_pad[:T] = seq.astype(np.int32).astype(bf16)  # ids < 256 are exact in bf16
    seq_pad = seq_pad.reshape(N_CORES, tok_pad)

    seq_i = np.asarray(seq, dtype=np.int64)
    counts = np.bincount(seq_i[(seq_i >= 0) & (seq_i < B)],
                         minlength=B)[:B].astype(np.float64)
    scale = (alpha.astype(np.float64)[None, :] / counts[:, None]).astype(np.float32)
    biasr = np.broadcast_to((bias.astype(np.float64) / N_CORES).astype(np.float32),
                            (B, D)).copy()

    has_bias = bool(np.any(biasr))
    key = (tok_pad, B, D, has_bias, U)
    if key not in _cache:
        _cache[key] = _build(tok_pad, B, D, has_bias)
    nc = _cache[key]

    in_maps = []
    for c in range(N_CORES):
        m = {"xhl": xhl[c], "seq": seq_pad[c], "scale": scale}
        if has_bias:
            m["biasr"] = biasr
        in_maps.append(m)

    trace = os.environ.get("BASS_KERNEL_TRACE", "") == "1"
    run_kwargs = {}
    if trace:
        import sys
        import types

        import antenv
        from trn_agent_boot.trn_boot import _ntff_profile_via_ctypes

        if "antenv.axon_hooks" not in sys.modules:
            mod = types.ModuleType("antenv.axon_hooks")
            hook = _ntff_profile_via_ctypes("/opt/axon/libaxon_pjrt.so")
            mod.get_axon_ntff_profile_hook = lambda: hook
            sys.modules["antenv.axon_hooks"] = mod
            antenv.axon_hooks = mod
        from concourse import bass_utils as _bu

        _bu.upload_artifacts = lambda tmpdir: "local://skipped"
        run_kwargs = {"trace": True, "trace_cores": [0]}

    from concourse.bass_utils import run_bass_kernel_spmd

    res = run_bass_kernel_spmd(nc, in_maps, core_ids=list(range(N_CORES)),
                               **run_kwargs)
    last_results = res

    pooled = np.zeros((B, D), dtype=np.float32)
    for r in res.results:
        pooled += r["out"]
    return (pooled, None)
